# revision 7
# baseline (speedup 1.0000x reference)
"""Trainium2 kernel for nn_DWT_Features.

The reference applies a 3-level db4 DWT along the time axis of every
(batch, pixel) signal, then contracts the coefficients with a full-volume
conv kernel and applies LeakyReLU.  The DWT is a linear map sig[64] ->
coeffs[84], so the whole network collapses to a single GEMM:

    out = leaky_relu(X @ W_eff + b),  X: [B, 4096], W_eff: [4096, 64]

where W_eff[(t,h,w), k] = sum_c M[t, c] * conv_w[k, c, h, w] and M is the
64x84 DWT matrix (computed here in numpy, folded on host - O(22M) flops).

Sharding: pure data parallel, batch split across 8 cores (1024 rows each).

Per-core kernel (all fp32):
  - X rows are loaded natively [128 batch, 1024 feat] (contiguous DMA),
  - transposed on the tensor engine via identity-matmul into PSUM,
  - PSUM -> SBUF copies alternate between Vector and Scalar engines,
  - the GEMM accumulates C.T[64, 512] = sum_k W_k.T @ Xt_k in PSUM with
    float32r operands (full fp32 bits, 4x PE streaming rate),
  - bias + LeakyReLU applied on-chip, C.T stored; host transposes back.
"""

import os
import sys

import numpy as np

if "/opt/trn_rl_repo" not in sys.path:
    sys.path.insert(0, "/opt/trn_rl_repo")

B, T, HW, K = 8192, 64, 8, 64
NCORES = 8
BPC = B // NCORES  # 1024 batch rows per core
F = T * HW * HW  # 4096 contracted features
NEG_SLOPE = 0.001
FILT_LEN = 8
BBLK = 512  # batch columns per PSUM accumulator
CHUNK = 1024  # xnat chunk free-dim (8 k-slices of 128)

DB4_LO = np.array(
    [-0.010597401784997278, 0.032883011666982945, 0.030841381835986965,
     -0.18703481171888114, -0.02798376941698385, 0.6308807679295904,
     0.7148465705525415, 0.23037781330885523], dtype=np.float64)
DB4_HI = np.array(
    [-0.23037781330885523, 0.7148465705525415, -0.6308807679295904,
     -0.02798376941698385, 0.18703481171888114, 0.030841381835986965,
     0.032883011666982945, -0.010597401784997278], dtype=np.float64)


def _afb1d(x):
    # numpy mirror of the reference: reflect pad, correlate with reversed
    # filters, stride 2.  x: [N, n] float64.
    n = x.shape[-1]
    out = (n + FILT_LEN - 1) // 2
    p = 2 * (out - 1) - n + FILT_LEN
    xp = np.pad(x, ((0, 0), (p // 2, (p + 1) // 2)), mode="reflect")
    idx = 2 * np.arange(out)[:, None] + np.arange(FILT_LEN)[None, :]
    win = xp[:, idx]  # [N, out, 8]
    return win @ DB4_LO[::-1], win @ DB4_HI[::-1]


def _dwt_matrix():
    # M [64, 84] with coeffs = sig @ M (image of the identity basis).
    lo, his = np.eye(T, dtype=np.float64), []
    for _ in range(3):
        lo, hi = _afb1d(lo)
        his.append(hi)
    return np.concatenate([lo] + his, axis=-1)


def _build_bass():
    import concourse.bacc as bacc
    import concourse.mybir as mybir
    import concourse.tile as tile
    from concourse import masks

    f32 = mybir.dt.float32
    f32r = mybir.dt.float32r
    Ident = mybir.ActivationFunctionType.Identity
    Alu = mybir.AluOpType

    nc = bacc.Bacc("TRN2", target_bir_lowering=False, debug=False)
    x_d = nc.dram_tensor("x", [BPC, F], f32, kind="ExternalInput").ap()
    w_d = nc.dram_tensor("w", [128, (F // 128) * K], f32, kind="ExternalInput").ap()
    b_d = nc.dram_tensor("b", [K, 1], f32, kind="ExternalInput").ap()
    o_d = nc.dram_tensor("out", [K, BPC], f32, kind="ExternalOutput").ap()

    NKC = F // 128  # 32 contraction chunks
    NB = BPC // BBLK  # 2 batch blocks
    NJ = BBLK // 128  # 4 partition groups per batch block
    NC_CHUNK = F // CHUNK  # 4 load chunks per xnat row-group

    with tile.TileContext(nc) as tc:
        with (
            tc.tile_pool(name="const", bufs=1) as constp,
            tc.tile_pool(name="xnat", bufs=10) as xpool,
            tc.tile_pool(name="xt", bufs=4) as xtp,
            tc.tile_pool(name="outs", bufs=4) as outp,
            tc.tile_pool(name="pt", bufs=3, space="PSUM") as ptp,
            tc.tile_pool(name="acc", bufs=2, space="PSUM") as accp,
        ):
            wsb_raw = constp.tile([128, NKC * K], f32)
            nc.sync.dma_start(wsb_raw[:], w_d[:])
            # fp32r operands must be produced rounded; one-time convert.
            wsb = constp.tile([128, NKC * K], f32r)
            nc.vector.tensor_copy(wsb[:], wsb_raw[:])
            bias = constp.tile([K, 1], f32)
            nc.sync.dma_start(bias[:], b_d[:])
            ident = constp.tile([128, 128], f32)
            masks.make_identity(nc, ident[:])

            for bb in range(NB):
                xn = {}
                # c-major issue order: the 4 row-groups of chunk c land
                # before chunk c+1, so k-slices unblock in k order.
                for c in range(NC_CHUNK):
                    for j in range(NJ):
                        t = xpool.tile([128, CHUNK], f32, name=f"xn{bb}_{c}_{j}",
                                       tag="xn")
                        r0 = bb * BBLK + j * 128
                        nc.sync.dma_start(
                            t[:], x_d[r0:r0 + 128, c * CHUNK:(c + 1) * CHUNK])
                        xn[(c, j)] = t

                acc = accp.tile([K, BBLK], f32)
                for k in range(NKC):
                    c, col = k // 8, (k % 8) * 128
                    pt = ptp.tile([128, BBLK], f32)
                    for j in range(NJ):
                        nc.tensor.matmul(
                            pt[:, j * 128:(j + 1) * 128],
                            xn[(c, j)][:, col:col + 128],
                            ident[:],
                            is_transpose=True,
                            start=(j == 0),
                            stop=(j == NJ - 1),
                        )
                    xt = xtp.tile([128, BBLK], f32r)
                    if k % 2 == 0:
                        nc.vector.tensor_copy(xt[:], pt[:])
                    else:
                        nc.scalar.copy(xt[:], pt[:])
                    nc.tensor.matmul(
                        acc[:],
                        wsb[:, k * K:(k + 1) * K],
                        xt[:],
                        start=(k == 0),
                        stop=(k == NKC - 1),
                    )

                t1 = outp.tile([K, BBLK], f32)
                nc.scalar.activation(t1[:], acc[:], Ident, bias=bias[:])
                ot = outp.tile([K, BBLK], f32)
                nc.vector.scalar_tensor_tensor(
                    ot[:], t1[:], NEG_SLOPE, t1[:], op0=Alu.mult, op1=Alu.max)
                nc.sync.dma_start(o_d[:, bb * BBLK:(bb + 1) * BBLK], ot[:])
    nc.compile()
    return nc


def _prep_inputs(x, conv_w, conv_b):
    M = _dwt_matrix()  # [64, 84]
    # W_eff[(t,h,w), k] = sum_c M[t,c] conv_w[k,c,h,w]
    w_eff = np.einsum("tc,kchw->thwk", M, conv_w.astype(np.float64))
    w2 = np.ascontiguousarray(w_eff.reshape(F, K)).astype(np.float32)
    # SBUF layout: wprep[p, k*K + n] = w2[k*128 + p, n]
    wprep = np.ascontiguousarray(
        w2.reshape(F // 128, 128, K).transpose(1, 0, 2).reshape(128, -1))
    bias = np.ascontiguousarray(
        np.asarray(conv_b, dtype=np.float32).reshape(K, 1))
    xf = np.ascontiguousarray(np.asarray(x, dtype=np.float32).reshape(B, F))
    return xf, wprep, bias


def kernel(x, conv_w, conv_b):
    from concourse.bass_utils import run_bass_kernel_spmd

    xf, wprep, bias = _prep_inputs(x, conv_w, conv_b)
    nc = _build_bass()
    in_maps = [
        {"x": xf[c * BPC:(c + 1) * BPC], "w": wprep, "b": bias}
        for c in range(NCORES)
    ]
    res = run_bass_kernel_spmd(nc, in_maps, list(range(NCORES)))
    out = np.concatenate([r["out"].T for r in res.results], axis=0)
    return np.ascontiguousarray(out, dtype=np.float32)
